# revision 6
# baseline (speedup 1.0000x reference)
"""Trainium2 Bass kernel for chunked-prefill GQA attention with KV cache.

Problem (hardcoded shapes): N=2048 new queries, 32 q-heads / 8 kv-heads (GQA),
head_dim=128, kv cache pre-filled with 2048 tokens, new k/v appended at slots
2048..4095, offset-causal mask, softmax, out = attn @ v.

Sharding: tensor-parallel over heads. Core g handles kv-head g and q-heads
4g..4g+3. Embarrassingly parallel; no collectives.

Per-core kernel layout (all matmuls bf16, fp32 PSUM accumulate):
  - Q^T [128=hd, 2048] per head and K^T [128=hd, 4096] via cast-DMA +
    DMA-transpose.  V natural [128=key, kb, 128+1] with a ones column; the
    PV matmul yields out-rows and the softmax denominator in one pass.
  - Scores computed transposed, S^T [128 keys, QCW queries] per key block.
  - exp is split across TWO engines: most batches go through the scalar
    engine (table exp), but selected non-diagonal batches are computed on
    the vector engine with a two-term ripple-cancelled Schraudolph exp2:
        V1 = int16(s*A + B); V2 = V1 + 64
        pt = bf16bits(V1) + W * bf16bits(V2)      (|rel err| <= 1.0%)
    The int16 bits ARE the bf16 result (exponent-field arithmetic), so the
    three DVE ops are mult-add (fp32->int16), +64 (int16, 2x mode), and one
    scalar_tensor_tensor combine (bf16, 2x mode).  The scalar engine is the
    1 elem/cycle/lane bottleneck otherwise; this rebalances ~25% of exp
    work onto the otherwise idle DVE at ~0.5% extra local error on ~25% of
    the probability mass.
  - Offloaded batches' PV matmuls are delayed two batch slots so the PE
    never waits on the slower DVE path; PSUM accumulation order within a
    group is irrelevant (delay never crosses the group's last batch).
  - Causal masks: multiplicative bf16 masks on the GPSIMD engine (only the
    last call of each group has diagonal blocks; those stay on the ACT path).
  - PSUM: 2 score bufs x 2 banks + 2x2 out accumulator banks (ping-pong
    across (h, qc) groups so the epilogue never blocks the next group).
"""

import math

import numpy as np

N_Q = 2048
CHUNK_START = 2048
T_KEYS = 4096
H = 32
KVH = 8
HQ = H // KVH  # q heads per core
HD = 128
SCALE = 1.0 / math.sqrt(HD)
N_CORES = 8

QCW = 256  # query-chunk width (moving free dim of the QK^T matmul)
KBATCH = 4  # key blocks per exp() batch (score tile = 2 PSUM banks)
KB = T_KEYS // 128  # 32 key blocks
VW = HD + 1  # V row width incl. ones column
K_CHUNKS = [16, 16]  # key-block chunking for K^T/V loads
PT_BUFS = 4
PTI_BUFS = 2
OSB_BUFS = 2
DEN_BUFS = 8
SC_BUFS = 2
SCD_BUFS = 1
OUTPS_BUFS = 1

# DVE exp2 constants: V = int16(s*EXP_A + EXP_B); pt = bf(V) + EXP_W*bf(V+64)
LOG2E = math.log2(math.e)
EXP_A = SCALE * LOG2E * 128.0
EXP_B = 127.0 * 128.0 - 134.95 + 0.5  # +0.5 hedges trunc-vs-round convert
EXP_W = 0.7075


def _build_nc(reps: int = 1):
    import concourse.bacc as bacc
    import concourse.mybir as mybir
    import concourse.tile as tile

    fp32 = mybir.dt.float32
    bf16 = mybir.dt.bfloat16
    i16 = mybir.dt.int16

    nc = bacc.Bacc("TRN2", target_bir_lowering=False, debug=False,
                   num_devices=N_CORES)

    q_in = nc.dram_tensor("q", [N_Q, HQ, HD], bf16, kind="ExternalInput")
    k_in = nc.dram_tensor("k", [T_KEYS, HD], bf16, kind="ExternalInput")
    v_in = nc.dram_tensor("v", [T_KEYS, HD], bf16, kind="ExternalInput")
    out = nc.dram_tensor("out", [N_Q, HQ, HD], fp32, kind="ExternalOutput")

    n_qc = N_Q // QCW
    chunk_of = {}  # kb -> (chunk index, offset within chunk)
    _kb = 0
    for ci, w in enumerate(K_CHUNKS):
        for o in range(w):
            chunk_of[_kb] = (ci, o)
            _kb += 1
    assert _kb == KB

    with tile.TileContext(nc) as tc:
        with (
            tc.tile_pool(name="dram", bufs=1, space="DRAM") as dram,
            tc.tile_pool(name="const", bufs=1) as const,
            tc.tile_pool(name="pt", bufs=PT_BUFS) as ptpool,
            tc.tile_pool(name="pti", bufs=PTI_BUFS) as ptipool,
            tc.tile_pool(name="osb", bufs=OSB_BUFS) as opool,
            tc.tile_pool(name="den", bufs=DEN_BUFS) as denpool,
            tc.tile_pool(name="scps", bufs=SC_BUFS, space="PSUM") as scpool,
            tc.tile_pool(name="scdps", bufs=SCD_BUFS, space="PSUM") as scdpool,
            tc.tile_pool(name="outps", bufs=OUTPS_BUFS,
                         space="PSUM") as outpspool,
        ):
            # ---- transposed operands straight from bf16 DRAM inputs ----
            # order: first-needed first (kt0, qt0, v0 feed the first batches)
            kts, qts, vsbs = [], [], []
            kb0c = 0
            for c, w in enumerate(K_CHUNKS):
                r0, r1 = kb0c * 128, (kb0c + w) * 128
                kb0c += w
                ktc = const.tile([128, w * 128], bf16, name=f"kt{c}")
                nc.sync.dma_start_transpose(ktc[:], k_in.ap()[r0:r1, :])
                kts.append(ktc)
                if c == 0:
                    qtc = const.tile([128, N_Q], bf16, name="qt0")
                    nc.sync.dma_start_transpose(qtc[:], q_in.ap()[:, 0, :])
                    qts.append(qtc)
                # V natural layout with ones column: [key%128, kb, hd+1]
                vc = const.tile([128, w, VW], bf16, name=f"v{c}")
                nc.gpsimd.dma_start(
                    vc[:, :, 0:HD],
                    v_in.ap()[r0:r1, :].rearrange("(kb p) d -> p kb d", p=128),
                )
                nc.vector.memset(vc[:, :, HD:VW], 1.0)
                vsbs.append(vc)
            for h in range(1, HQ):
                qtc = const.tile([128, N_Q], bf16, name=f"qt{h}")
                nc.sync.dma_start_transpose(qtc[:], q_in.ap()[:, h, :])
                qts.append(qtc)

            def kt_sl(kb):
                ci, o = chunk_of[kb]
                return kts[ci][:, o * 128:(o + 1) * 128]

            def v_sl(kb):
                ci, o = chunk_of[kb]
                return vsbs[ci][:, o, :]

            # ---- causal masks: mask[j][r, c] = 1.0 if r <= c - 128*j ----
            masks = const.tile([128, QCW // 128, QCW], bf16)
            nc.vector.memset(masks[:], 1.0)
            for j in range(QCW // 128):
                nc.gpsimd.affine_select(
                    out=masks[:, j, :],
                    in_=masks[:, j, :],
                    compare_op=mybir.AluOpType.is_ge,
                    fill=0.0,
                    base=-128 * j,
                    pattern=[[1, QCW]],
                    channel_multiplier=-1,
                )

            # flat batch schedule over (head, q-chunk, key-block batch).
            # dve=True batches use the vector-engine exp2 path; they are
            # never the group's first call (PSUM start flag must execute
            # first chronologically) nor one of the last two (the extra PV
            # delay must stay inside the group; diagonal masks also live in
            # the last call).  DVE batches are spaced >=3 apart so the
            # in-order vector engine never holds a score buffer late.
            batches = []
            group_last = []  # flat batch index of each group's last batch
            for h in range(HQ):
                for qc in range(n_qc):
                    n_kb = min(KB,
                               (CHUNK_START + (qc + 1) * QCW - 1) // 128 + 1)
                    n_calls = -(-n_kb // KBATCH)
                    base, extra = divmod(n_kb, n_calls)
                    kb0 = 0
                    for ci in range(n_calls):
                        bsz = base + (1 if ci < extra else 0)
                        dve = ci in (1, 4) and ci <= n_calls - 3
                        batches.append((h, qc, kb0, bsz, n_kb, dve))
                        kb0 += bsz
                    group_last.append(len(batches) - 1)
            group_last_set = set(group_last)

            n_batches = len(batches)

            def body():
                sc_tiles = {}
                pt_tiles = {}
                outs_of = {}  # group key (h, qc) -> PSUM accumulators
                pv_pending = {}  # emit slot -> list of batch indices

                def emit_qk(bi):
                    h, qc, kb0, bsz, n_kb, dve = batches[bi]
                    pool = scdpool if dve else scpool
                    sc = pool.tile([128, KBATCH, QCW], fp32,
                                   name="sc", tag="scd" if dve else "sc")
                    sc_tiles[bi] = sc
                    for b in range(bsz):
                        kb = kb0 + b
                        # the group's last block only sees the second half
                        # of the q-chunk (off is always exactly -128); the
                        # first half is fully causal-masked garbage that the
                        # j=1 mask zeroes and whose PV is skipped
                        q0 = 128 if kb == n_kb - 1 else 0
                        nc.tensor.matmul(
                            sc[:, b, q0:QCW],
                            lhsT=kt_sl(kb),
                            rhs=qts[h][:, qc * QCW + q0:(qc + 1) * QCW],
                            start=True, stop=True,
                        )

                def emit_pv(bi):
                    h, qc, kb0, bsz, n_kb, dve = batches[bi]
                    pt = pt_tiles.pop(bi)
                    outs = outs_of[(h, qc)]
                    for b in range(bsz):
                        kb = kb0 + b
                        for sq in range(QCW // 128):
                            if sq == 0 and kb == n_kb - 1:
                                continue  # fully masked: contributes zero
                            nc.tensor.matmul(
                                outs[sq][:],
                                lhsT=pt[:, b, sq * 128:(sq + 1) * 128],
                                rhs=v_sl(kb),
                                start=(kb == 0),
                                stop=(kb == n_kb - 1
                                      or (sq == 0 and kb == n_kb - 2)),
                            )
                    if bi in group_last_set:
                        # epilogue: normalize by the ones-column sum, store
                        osb = opool.tile([128, QCW // 128, HD], fp32,
                                         name="osb", tag="osb")
                        for sq in range(QCW // 128):
                            den = denpool.tile([128, 1], fp32,
                                               name="den", tag="den")
                            nc.vector.reciprocal(den[:], outs[sq][:, HD:VW])
                            nc.vector.tensor_scalar_mul(
                                osb[:, sq, :], outs[sq][:, 0:HD], den[:])
                        nc.sync.dma_start(
                            out.ap()[qc * QCW:(qc + 1) * QCW, h, :]
                               .rearrange("(s p) d -> p s d", p=128),
                            osb[:],
                        )
                        del outs_of[(h, qc)]

                emit_qk(0)
                for bi in range(n_batches + 3):
                    if bi < n_batches:
                        h, qc, kb0, bsz, n_kb, dve = batches[bi]
                        if kb0 == 0:
                            # one PSUM bank per accumulation group; the
                            # delayed epilogue finishes before the next
                            # group's first (also delayed) PV needs the bank
                            outs_of[(h, qc)] = [
                                outpspool.tile([128, VW], fp32,
                                               tag=f"out{i}", name=f"out{i}")
                                for i in range(QCW // 128)
                            ]
                        sc = sc_tiles.pop(bi)
                        pt = ptpool.tile([128, KBATCH, QCW], bf16,
                                         name="pt", tag="pt")
                        pt_tiles[bi] = pt
                        if dve:
                            # vector-engine exp2: only the first op touches
                            # the PSUM score tile, so the sc buffer frees
                            # early; ops 2-3 run on int16/bf16 bits
                            p1 = ptipool.tile([128, KBATCH, QCW], i16,
                                              name="p1", tag="p1")
                            p2 = ptipool.tile([128, KBATCH, QCW], i16,
                                              name="p2", tag="p2")
                            nc.vector.tensor_scalar(
                                p1[:, :bsz, :], sc[:, :bsz, :],
                                EXP_A, EXP_B,
                                mybir.AluOpType.mult, mybir.AluOpType.add,
                            )
                            nc.vector.tensor_scalar_add(
                                p2[:, :bsz, :], p1[:, :bsz, :], 64)
                            nc.vector.scalar_tensor_tensor(
                                pt[:, :bsz, :],
                                p2[:, :bsz, :].bitcast(bf16),
                                EXP_W,
                                p1[:, :bsz, :].bitcast(bf16),
                                mybir.AluOpType.mult, mybir.AluOpType.add,
                            )
                        else:
                            nc.scalar.activation(
                                pt[:, :bsz, :], sc[:, :bsz, :],
                                mybir.ActivationFunctionType.Exp,
                                scale=SCALE,
                            )
                        if bi + 1 < n_batches:
                            emit_qk(bi + 1)
                        for b in range(bsz):
                            kb = kb0 + b
                            off = CHUNK_START + qc * QCW - kb * 128
                            if off < 128:  # diagonal block: mask on gpsimd
                                j = -off // 128 if off < 0 else 0
                                nc.gpsimd.tensor_mul(
                                    pt[:, b, :], pt[:, b, :], masks[:, j, :])
                        # PV delayed one slot (two for DVE batches) so the
                        # in-order PE always has the next QK batch queued
                        # ahead of any pt-dependency wait
                        pv_pending.setdefault(bi + (2 if dve else 1),
                                              []).append(bi)
                    # flush: non-DVE (higher bi) first
                    for pbi in sorted(pv_pending.pop(bi, ()), reverse=True):
                        emit_pv(pbi)

            if reps == 1:
                body()
            else:
                # timing-only loop; hint back-edge branch targets so the
                # IRAM refetch (~4us for >256-inst bodies) is prefetched
                with tc.For_i(0, reps, 1, hint_engines=(
                        mybir.EngineType.PE,
                        mybir.EngineType.Activation,
                        mybir.EngineType.DVE,
                        mybir.EngineType.SP,
                        mybir.EngineType.Pool)):
                    body()

    nc.compile()
    return nc


_NC_CACHE: dict = {}


def _get_nc(reps: int = 1):
    if reps not in _NC_CACHE:
        _NC_CACHE[reps] = _build_nc(reps)
    return _NC_CACHE[reps]


def _shard_inputs(q, k, v, k_cache, v_cache, slot_mapping, chunk_start):
    import ml_dtypes
    bf = ml_dtypes.bfloat16

    cs = int(chunk_start)
    n = q.shape[0]
    sm = np.asarray(slot_mapping)
    q = np.asarray(q, dtype=np.float32)
    k = np.asarray(k, dtype=np.float32)
    v = np.asarray(v, dtype=np.float32)
    k_cache = np.asarray(k_cache, dtype=np.float32)
    v_cache = np.asarray(v_cache, dtype=np.float32)

    if np.array_equal(sm, np.arange(n, dtype=sm.dtype) + cs):
        k_eff = np.concatenate([k_cache[:cs], k], axis=0)  # [T, KVH, HD]
        v_eff = np.concatenate([v_cache[:cs], v], axis=0)
    else:  # general path: honor arbitrary slot mappings
        kc = k_cache.copy()
        vc = v_cache.copy()
        kc[sm] = k
        vc[sm] = v
        k_eff = kc[:cs + n]
        v_eff = vc[:cs + n]

    k_eff = k_eff.astype(bf)
    v_eff = v_eff.astype(bf)
    q = q.astype(bf)

    in_maps = []
    for g in range(N_CORES):
        in_maps.append({
            "q": np.ascontiguousarray(q[:, g * HQ:(g + 1) * HQ, :]),
            "k": np.ascontiguousarray(k_eff[:, g, :]),
            "v": np.ascontiguousarray(v_eff[:, g, :]),
        })
    return in_maps


def kernel(q, k, v, k_cache, v_cache, slot_mapping, chunk_start, **_unused):
    from concourse import bass_utils

    in_maps = _shard_inputs(q, k, v, k_cache, v_cache, slot_mapping,
                            chunk_start)
    nc = _get_nc()
    res = bass_utils.run_bass_kernel_spmd(nc, in_maps,
                                          core_ids=list(range(N_CORES)))
    return np.concatenate([res.results[g]["out"] for g in range(N_CORES)],
                          axis=1)
